# revision 18
# baseline (speedup 1.0000x reference)
# Trainium2 Bass kernel for nn_CCE_Model_77936476553266
# ResNet50[:~layer2] backbone + FC head + stereographic projection + CCE loss.
# Data-parallel over 8 NeuronCores: 256 samples/core. bf16 matmuls, fp32 PSUM.
#
# 64-channel tensors use a "paired" layout: [128, b/2, H, W] where partitions
# 0-63 hold ch0-63 of the first half of the samples and partitions 64-127 hold
# ch0-63 of the second half.  Matmuls on these run as concurrent tile_position
# pairs (2x PE array utilization) and evacuations run at full 128-lane width.
import math
import os
from contextlib import ExitStack

import numpy as np
import ml_dtypes

import concourse.bass as bass
from concourse import bacc
import concourse.mybir as mybir
import concourse.tile as tile
from concourse.bass_utils import run_bass_kernel_spmd

BF = ml_dtypes.bfloat16
bf16 = mybir.dt.bfloat16
f32 = mybir.dt.float32
AF = mybir.ActivationFunctionType
ALU = mybir.AluOpType

NCORES = 8
BTOT = 2048
B = BTOT // NCORES    # per-core batch = 256
BQ = 32               # main loop batch chunk
H2 = BQ // 2          # samples per paired half = 16
EPS_BN = 1e-5

_PROG = None  # cached program


# ---------------------------------------------------------------- host prep
def _fold(w, bn):
    inv = (bn['g'] / np.sqrt(bn['v'] + EPS_BN)).astype(np.float64)
    beta = (bn['b'] - bn['m'] * inv).astype(np.float32)
    return (np.asarray(w, np.float64) * inv[:, None, None, None]).astype(np.float32), beta


def _pack_conv(w, dup=False):
    """w [O, I, kh, kw] (fp32, BN-folded) -> lhsT [Kp, n_kc, n_taps, n_mc, M].
    dup=True: K<=64 duplicated to rows 64-127 (for paired-input matmul tiles)."""
    O, I, kh, kw = w.shape
    n_kc = (I + 127) // 128
    Kp = min(I, 128)
    n_mc = (O + 127) // 128
    M = min(O, 128)
    a = np.zeros((Kp, n_kc, kh * kw, n_mc, M), np.float32)
    for kc in range(n_kc):
        ksz = min(128, I - kc * 128)
        for t in range(kh * kw):
            khi, kwi = divmod(t, kw)
            for mc in range(n_mc):
                msz = min(128, O - mc * 128)
                a[:ksz, kc, t, mc, :msz] = w[mc * 128:mc * 128 + msz,
                                             kc * 128:kc * 128 + ksz, khi, kwi].T
    if dup:
        assert I <= 64
        a = np.concatenate([a, a], axis=0)  # rows 64-127 = copy
    return a.astype(BF)


def _pack_bias(beta, dup=False):
    C = beta.shape[0]
    n_mc = (C + 127) // 128
    M = min(C, 128)
    b = np.zeros((M, n_mc), np.float32)
    for mc in range(n_mc):
        msz = min(128, C - mc * 128)
        b[:msz, mc] = beta[mc * 128:mc * 128 + msz]
    if dup:
        assert C <= 64
        b = np.concatenate([b, b], axis=0)
    return b


def _host_arrays(params):
    """Shared (replicated) input arrays: weights, biases, constants."""
    P = params
    arrs = {}

    # conv1: im2col lhsT [147, 64] -> padded [128, 2, 1, 1, 64]
    w1, beta1 = _fold(np.asarray(P['conv1'], np.float32),
                      {k: np.asarray(v, np.float32) for k, v in P['bn1'].items()})
    lh = w1.transpose(1, 2, 3, 0).reshape(147, 64)  # k = c*49+kh*7+kw
    a = np.zeros((128, 2, 1, 1, 64), np.float32)
    a[:, 0, 0, 0, :] = lh[:128]
    a[:19, 1, 0, 0, :] = lh[128:]
    arrs['c1_w'] = a.astype(BF)
    arrs['c1_b'] = _pack_bias(beta1, dup=True)

    def blockw(p, pref, first):
        bn = lambda dd: {k: np.asarray(v, np.float32) for k, v in dd.items()}
        w1_, b1_ = _fold(np.asarray(p['w1'], np.float32), bn(p['bn1']))
        w2_, b2_ = _fold(np.asarray(p['w2'], np.float32), bn(p['bn2']))
        w3_, b3_ = _fold(np.asarray(p['w3'], np.float32), bn(p['bn3']))
        l1 = pref.startswith('l1')
        # layer1: w1 paired-K for b0 ('pp'), plain for b1/b2 ('bp'); w2/w3 dup.
        arrs[pref + '_w1'] = _pack_conv(w1_, dup=(l1 and first))
        arrs[pref + '_b1'] = _pack_bias(b1_, dup=l1)
        arrs[pref + '_w2'] = _pack_conv(w2_, dup=l1)
        arrs[pref + '_b2'] = _pack_bias(b2_, dup=l1)
        arrs[pref + '_w3'] = _pack_conv(w3_, dup=l1)
        if 'wd' in p:
            wd_, bd_ = _fold(np.asarray(p['wd'], np.float32), bn(p['bnd']))
            arrs[pref + '_wd'] = _pack_conv(wd_, dup=l1)
            arrs[pref + '_b3'] = _pack_bias(b3_ + bd_)
        else:
            arrs[pref + '_b3'] = _pack_bias(b3_)

    for i, blk in enumerate(P['layer1']):
        blockw(blk, f'l1b{i}', i == 0)
    for i, blk in enumerate(P['layer2']):
        blockw(blk, f'l2b{i}', i == 0)

    # fc: [512, 8192]; feature k = c*16 + hw -> lhsT [128, 16(hw), 4(ci), 4(co), 128]
    fcw = np.asarray(P['fc_w'], np.float32)
    t = fcw.reshape(512, 512, 16)
    a = np.zeros((128, 4, 16, 4, 128), np.float32)
    for hw in range(16):
        for ci in range(4):
            for co in range(4):
                a[:, ci, hw, co, :] = t[co * 128:(co + 1) * 128,
                                        ci * 128:(ci + 1) * 128, hw].T
    arrs['fc_w'] = a.astype(BF)
    arrs['fc_b'] = _pack_bias(np.asarray(P['fc_b'], np.float32))

    fc1 = np.asarray(P['fc1_w'], np.float32)
    a = np.zeros((128, 4, 1, 2, 128), np.float32)
    for kc in range(4):
        for mc in range(2):
            a[:, kc, 0, mc, :] = fc1[mc * 128:(mc + 1) * 128,
                                     kc * 128:(kc + 1) * 128].T
    arrs['fc1_w'] = a.astype(BF)
    arrs['fc1_b'] = _pack_bias(np.asarray(P['fc1_b'], np.float32))

    fc2 = np.asarray(P['fc2_w'], np.float32)
    arrs['fc2_w0'] = np.ascontiguousarray(fc2[:, 0:128].T)
    arrs['fc2_w1'] = np.ascontiguousarray(fc2[:, 128:256].T)
    arrs['fc2_w2'] = np.ascontiguousarray(fc2[:, 256:257].T)

    arrs['identb'] = np.eye(128, dtype=np.float32).astype(BF)
    arrs['identf'] = np.eye(128, dtype=np.float32)
    arrs['ones128'] = np.ones((128, 1), np.float32)
    arrs['ones1x'] = np.ones((1, 128), np.float32)
    return arrs


def _host_percore(x, labels):
    """Per-core arrays: conv1 im2col (bf16) and one-hot labels (f32)."""
    x = np.asarray(x, np.float32)
    labels = np.asarray(labels)
    xp = np.zeros((BTOT, 3, 38, 38), np.float32)
    xp[:, :, 3:35, 3:35] = x
    v = np.lib.stride_tricks.sliding_window_view(xp, (7, 7), axis=(2, 3))
    v = v[:, :, ::2, ::2, :, :]                        # [B,3,16,16,7,7]
    im = v.transpose(1, 4, 5, 0, 2, 3).reshape(147, BTOT, 256).astype(BF)
    per = []
    for c in range(NCORES):
        sl = im[:, c * B:(c + 1) * B, :]
        im0 = np.ascontiguousarray(sl[:128]).reshape(128, B * 256)
        im1 = np.ascontiguousarray(sl[128:147]).reshape(19, B * 256)
        lab = labels[c * B:(c + 1) * B].astype(np.int64)
        oh = np.zeros((128, 2, 257), np.float32)
        for bc in range(2):
            for p in range(128):
                oh[p, bc, lab[bc * 128 + p]] = 1.0
        per.append({'im0': im0, 'im1': im1, 'oneh': oh})
    return per


# ---------------------------------------------------------------- device build
def _valid_range(d, s, OH, H):
    o0 = max(0, math.ceil(-d / s))
    o1 = min(OH, (H - 1 - d) // s + 1)
    return None if o1 <= o0 else (o0, o1)


def _regions(taps, s, OH, OW, H, W):
    out = []
    for (t, dh, dw) in taps:
        r = _valid_range(dh, s, OH, H)
        c = _valid_range(dw, s, OW, W)
        if r and c:
            out.append((t, dh, dw, r[0], r[1], c[0], c[1]))
    return out


def _rsl(o0, o1, s, d):
    return slice(o0 * s + d, (o1 - 1) * s + d + 1, s)


class _St:
    pass


def _flat(ap):
    n = len(ap.shape) - 1
    if n <= 1:
        return ap
    names = ' '.join(f'd{i}' for i in range(n))
    return ap.rearrange(f'p {names} -> p ({names})')


def _evac(st, ps, out, bias, relu):
    nc = st.nc
    i_ = _flat(ps)
    o_ = _flat(out)
    st.ev = (st.ev + 1) % 2
    if st.ev == 0:
        func = AF.Relu if relu else AF.Identity
        if bias is None and not relu:
            nc.scalar.copy(o_, i_)
        else:
            nc.scalar.activation(o_, i_, func,
                                 bias=bias if bias is not None else 0.0, scale=1.0)
    else:
        if bias is None:
            if relu:
                nc.vector.tensor_scalar(o_, i_, 0.0, None, ALU.max)
            else:
                nc.vector.tensor_copy(o_, i_)
        elif relu:
            nc.vector.tensor_scalar(o_, i_, bias, 0.0, ALU.add, ALU.max)
        else:
            nc.vector.tensor_scalar(o_, i_, bias, None, ALU.add)


def _emit_mms(nc, jobs, n_start=1):
    n = len(jobs)
    for i, (o, lh, r, tp) in enumerate(jobs):
        nc.tensor.matmul(o, lh, r, start=(i < n_start), stop=(i == n - 1),
                         tile_position=tp, skip_group_check=True)


def _conv_pp(st, *, name, w, bias, in_, out, taps, s, H, W, OH, OW, relu=True):
    """paired -> paired, K<=64, M<=64.  Diagonal concurrent tile pairs."""
    nc = st.nc
    regs = _regions(taps, s, OH, OW, H, W)
    nb = in_.shape[1]            # samples per half
    bank = max(1, 512 // (OH * OW))
    ps = st.psA.tile([128, nb, OH, OW], f32, tag='ps', name=f'{name}ps')
    for b0 in range(0, nb, bank):
        b1 = min(b0 + bank, nb)
        jobs = []
        for (t, dh, dw, oh0, oh1, ow0, ow1) in regs:
            rsl, csl = _rsl(oh0, oh1, s, dh), _rsl(ow0, ow1, s, dw)
            jobs.append((ps[0:64, b0:b1, oh0:oh1, ow0:ow1],
                         w[0:64, 0, t, 0, :], in_[0:64, b0:b1, rsl, csl], (0, 0)))
            jobs.append((ps[64:128, b0:b1, oh0:oh1, ow0:ow1],
                         w[64:128, 0, t, 0, :], in_[64:128, b0:b1, rsl, csl],
                         (64, 64)))
        _emit_mms(nc, jobs, n_start=2)
    _evac(st, ps[:], out, bias, relu)


def _conv_bp(st, *, name, w, bias, kps, ins, ins_hi, out, relu=True):
    """plain (full-K chunks) -> paired 64ch out, 1x1 conv.
    ins: per-kc APs [kp, nlo, H, W] for the lower-half samples;
    ins_hi: same for upper-half samples."""
    nc = st.nc
    nb = ins[0].shape[1]
    OHW = ins[0].shape[2] * ins[0].shape[3]
    bank = max(1, 512 // OHW)
    psn = max(1, 1024 // OHW)    # samples per psum tile (per half)
    for g0 in range(0, nb, psn):
        g1 = min(g0 + psn, nb)
        ps = st.psA.tile([128, g1 - g0, ins[0].shape[2], ins[0].shape[3]], f32,
                         tag='ps', name=f'{name}ps')
        for b0 in range(g0, g1, bank):
            b1 = min(b0 + bank, g1)
            jobs = []
            for kc in range(len(ins)):
                jobs.append((ps[0:64, b0 - g0:b1 - g0], w[:kps[kc], kc, 0, 0, :],
                             ins[kc][:, b0:b1], (0, 0)))
                jobs.append((ps[64:128, b0 - g0:b1 - g0], w[:kps[kc], kc, 0, 0, :],
                             ins_hi[kc][:, b0:b1], (0, 64)))
            _emit_mms(nc, jobs, n_start=2)
        _evac(st, ps[:], out[:, g0:g1], bias, relu)


def _conv_pd(st, *, name, w, bias, in_, outs, s, H, W, OH, OW, extras=None,
             relu=True):
    """paired in (K<=64 dup weights) -> plain M-chunk outs.  1x1 convs only.
    outs: per-mc APs [128, 2*nb, OH, OW] (full chunk); extras(mc, half, b0, b1)
    -> [(lhsT, rhs, tp)] full-coverage accumulands (identity / wd)."""
    nc = st.nc
    nb = in_.shape[1]
    bank = max(1, 512 // (OH * OW))
    psn = max(1, 1024 // (OH * OW))
    n_mc = len(outs)
    for mc in range(n_mc):
        for half in range(2):
            p0 = 64 * half
            for g0 in range(0, nb, psn):
                g1 = min(g0 + psn, nb)
                ps = st.psA.tile([128, g1 - g0, OH, OW], f32, tag='ps',
                                 name=f'{name}ps')
                for b0 in range(g0, g1, bank):
                    b1 = min(b0 + bank, g1)
                    jobs = [(ps[:, b0 - g0:b1 - g0], w[p0:p0 + 64, 0, 0, mc, :],
                             in_[p0:p0 + 64, b0:b1, 0:(H - 1) * s + 1:s,
                                 0:(W - 1) * s + 1:s], (p0, 0))]
                    if extras is not None:
                        jobs += [(ps[:, b0 - g0:b1 - g0], lh, r, tp)
                                 for (lh, r, tp) in extras(mc, half, b0, b1)]
                    _emit_mms(nc, jobs)
                _evac(st, ps[:], outs[mc][:, half * nb + g0:half * nb + g1],
                      None if bias is None else bias[:, mc:mc + 1], relu)


def _conv_dd(st, *, name, w, bias, kps, ins, outs, taps, s, H, W, OH, OW,
             extras=None, relu=True):
    """plain -> plain (all dims >= 128 on at least one side)."""
    nc = st.nc
    regs = _regions(taps, s, OH, OW, H, W)
    nb = ins[0].shape[1]
    Mp = outs[0].shape[0]
    bank = max(1, 512 // (OH * OW))
    psn = max(1, 1024 // (OH * OW))
    for mc in range(len(outs)):
        for g0 in range(0, nb, psn):
            g1 = min(g0 + psn, nb)
            ps = st.psA.tile([Mp, g1 - g0, OH, OW], f32, tag='ps',
                             name=f'{name}ps')
            for b0 in range(g0, g1, bank):
                b1 = min(b0 + bank, g1)
                jobs = []
                for (t, dh, dw, oh0, oh1, ow0, ow1) in regs:
                    rsl, csl = _rsl(oh0, oh1, s, dh), _rsl(ow0, ow1, s, dw)
                    for kc in range(len(ins)):
                        jobs.append((ps[:, b0 - g0:b1 - g0, oh0:oh1, ow0:ow1],
                                     w[:kps[kc], kc, t, mc, :],
                                     ins[kc][:, b0:b1, rsl, csl], None))
                if extras is not None:
                    jobs += [(ps[:, b0 - g0:b1 - g0], lh, r, None)
                             for (lh, r) in extras(mc, b0, b1)]
                _emit_mms(nc, jobs)
            _evac(st, ps[:], outs[mc][:, g0:g1],
                  None if bias is None else bias[:, mc:mc + 1], relu)


TAPS3 = [(4, 0, 0)] + [(kh * 3 + kw, kh - 1, kw - 1)
                       for kh in range(3) for kw in range(3) if not (kh == 1 and kw == 1)]
TAP1 = [(0, 0, 0)]


def _build_program():
    nc = bacc.Bacc(debug=False)

    d = {}
    wspecs = {
        'c1_w': ([128, 2, 1, 1, 64], bf16), 'c1_b': ([128, 1], f32),
        'fc_w': ([128, 4, 16, 4, 128], bf16), 'fc_b': ([128, 4], f32),
        'fc1_w': ([128, 4, 1, 2, 128], bf16), 'fc1_b': ([128, 2], f32),
        'fc2_w0': ([128, 10], f32), 'fc2_w1': ([128, 10], f32),
        'fc2_w2': ([1, 10], f32),
        'identb': ([128, 128], bf16), 'identf': ([128, 128], f32),
        'ones128': ([128, 1], f32), 'ones1x': ([1, 128], f32),
        'im0': ([128, B * 256], bf16), 'im1': ([19, B * 256], bf16),
        'oneh': ([128, 2, 257], f32),
    }

    l1cfg = [(64, 64, 256, True), (256, 64, 256, False), (256, 64, 256, False)]
    l2cfg = [(256, 128, 512, True), (512, 128, 512, False),
             (512, 128, 512, False), (512, 128, 512, False)]
    for i, (cin, wdt, cout, down) in enumerate(l1cfg + l2cfg):
        pref = f'l1b{i}' if i < 3 else f'l2b{i - 3}'
        l1 = i < 3
        kp1 = 128 if (l1 and i == 0) else min(cin, 128)
        wspecs[pref + '_w1'] = ([kp1, (cin + 127) // 128, 1, 1, wdt], bf16)
        wspecs[pref + '_b1'] = ([wdt * 2 if l1 else wdt, 1], f32)
        wspecs[pref + '_w2'] = ([wdt * 2 if l1 else wdt, 1, 9, 1, wdt], bf16)
        wspecs[pref + '_b2'] = ([wdt * 2 if l1 else wdt, 1], f32)
        wspecs[pref + '_w3'] = ([wdt * 2 if l1 else wdt, 1, 1, cout // 128, 128], bf16)
        wspecs[pref + '_b3'] = ([128, cout // 128], f32)
        if down:
            kpd = 128 if l1 else min(cin, 128)
            wspecs[pref + '_wd'] = ([kpd, (cin + 127) // 128, 1, cout // 128, 128],
                                    bf16)

    for nm, (shp, dt) in wspecs.items():
        d[nm] = nc.declare_dram_parameter(nm, shp, dt, isOutput=False)
    d_out = nc.declare_dram_parameter('out_t', [10, B], f32, isOutput=True)
    d_loss = nc.declare_dram_parameter('lossp', [1, 1], f32, isOutput=True)
    DBG = bool(os.environ.get('KDBG'))
    ddbg = {}
    if DBG:
        for nm, shp in [('dbg_c1o', [128, 16 * 256]), ('dbg_pot', [128, 16 * 64]),
                        ('dbg_w1o', [128, 16 * 64]), ('dbg_w2o', [128, 16 * 64]),
                        ('dbg_o0', [128, 32 * 64]), ('dbg_o1', [128, 32 * 64])]:
            ddbg[nm] = nc.declare_dram_parameter(nm, shp, bf16, isOutput=True)

    st = _St()
    st.nc = nc
    st.ev = 0

    with tile.TileContext(nc) as tc:
        with ExitStack() as ctx:
            wp = ctx.enter_context(tc.tile_pool(name='wp', bufs=1))
            ag = ctx.enter_context(tc.tile_pool(name='ag', bufs=1))

            W = {}
            for nm, (shp, dt) in wspecs.items():
                if nm in ('im0', 'im1', 'fc_w'):
                    continue
                W[nm] = wp.tile(shp, dt, name=nm + 's')
                # conv1 weights on the sync queue (needed first); the rest on
                # gpsimd SWDGE queues so they don't delay the im2col loads.
                eng = nc.sync if nm in ('c1_w', 'c1_b') else nc.gpsimd
                eng.dma_start(out=_flat(W[nm]), in_=_flat(d[nm][:]))

            feat = [ag.tile([128, B, 16], bf16, name=f'feat{i}') for i in range(4)]

            # ---------------- phase A: stem + layer1 + layer2, chunks of BQ
            with ExitStack() as pa:
                apl = pa.enter_context(tc.tile_pool(name='apl', bufs=2))
                st.psA = pa.enter_context(
                    tc.tile_pool(name='psA', bufs=4, space='PSUM'))
                spx = ExitStack()
                sp = spx.enter_context(tc.tile_pool(name='sp', bufs=2))

                def stem(q):
                    # conv1 (host im2col) + maxpool, paired layout
                    q0 = q * BQ
                    imt = {}
                    for hf, nm in ((0, 'lo'), (1, 'hi')):
                        s0 = (q0 + hf * H2) * 256
                        i0 = sp.tile([128, H2, 16, 16], bf16, tag='im0' + nm,
                                     bufs=2, name='im0' + nm)
                        i1 = sp.tile([19, H2, 16, 16], bf16, tag='im1' + nm,
                                     bufs=2, name='im1' + nm)
                        nc.sync.dma_start(out=_flat(i0),
                                          in_=d['im0'][:, s0:s0 + H2 * 256])
                        nc.sync.dma_start(out=_flat(i1),
                                          in_=d['im1'][:, s0:s0 + H2 * 256])
                        imt[hf] = (i0, i1)
                    c1o = sp.tile([128, H2, 16, 16], bf16, tag='c1o', name='c1o')
                    _conv_bp(st, name='c1', w=W['c1_w'], bias=W['c1_b'][:, 0:1],
                             kps=[128, 19],
                             ins=[imt[0][0][:], imt[0][1][:]],
                             ins_hi=[imt[1][0][:], imt[1][1][:]], out=c1o[:])
                    # maxpool 3x3 s2 p1 (valid-region trick), paired full-width
                    cm = sp.tile([128, H2, 16, 8], bf16, tag='cm', name='cm')
                    pot = apl.tile([128, H2, 8, 8], bf16, tag='po', name='pot')
                    po = pot[:]
                    g = nc.vector
                    nc.scalar.copy(cm[:], c1o[:, :, :, 0:16:2])
                    g.tensor_max(cm[:, :, :, 1:8], cm[:, :, :, 1:8],
                                 c1o[:, :, :, 1:14:2])
                    g.tensor_max(cm[:], cm[:], c1o[:, :, :, 1:16:2])
                    nc.scalar.copy(po, cm[:, :, 0:16:2, :])
                    g.tensor_max(po[:, :, 1:8, :], po[:, :, 1:8, :],
                                 cm[:, :, 1:14:2, :])
                    g.tensor_max(po, po, cm[:, :, 1:16:2, :])
                    if DBG and q == 0:
                        nc.sync.dma_start(out=ddbg['dbg_c1o'][:],
                                          in_=_flat(c1o[:]))
                        nc.sync.dma_start(out=ddbg['dbg_pot'][:],
                                          in_=_flat(pot[:]))
                    return pot

                pots = {0: stem(0)}
                for q in range(B // BQ):
                    q0 = q * BQ
                    if q + 1 < B // BQ:
                        pots[q + 1] = stem(q + 1)
                    if q + 1 == B // BQ:
                        # all stems emitted: retire the stem pool; reuse its
                        # SBUF to preload fc weights during the last chunk so
                        # the PE rolls straight into the fc matmuls while warm
                        spx.close()
                        fcB = pa.enter_context(tc.tile_pool(name='fcB', bufs=1))
                        st.fcB = fcB
                        st.fcwt = []
                        for ci in range(4):
                            fw = fcB.tile([128, 16, 4, 128], bf16,
                                          name=f'fcw{ci}')
                            nc.sync.dma_start(out=_flat(fw),
                                              in_=_flat(d['fc_w'][:, ci]))
                            st.fcwt.append(fw)
                    po = pots.pop(q)[:]

                    # ---- layer1 (8x8), paired 64-ch intermediates
                    cur = None  # plain [128, BQ, 8, 8] chunk list after b0
                    for bi in range(3):
                        pref = f'l1b{bi}'
                        w1o = apl.tile([128, H2, 8, 8], bf16, tag='w1o8',
                                       name=pref + 'w1o')
                        if bi == 0:
                            _conv_pp(st, name=pref + 'w1', w=W[pref + '_w1'],
                                     bias=W[pref + '_b1'][:, 0:1], in_=po,
                                     out=w1o[:], taps=TAP1, s=1,
                                     H=8, W=8, OH=8, OW=8)
                        else:
                            _conv_bp(st, name=pref + 'w1', w=W[pref + '_w1'],
                                     bias=W[pref + '_b1'][:, 0:1], kps=[128, 128],
                                     ins=[c[:, 0:H2] for c in cur],
                                     ins_hi=[c[:, H2:BQ] for c in cur],
                                     out=w1o[:])
                        w2o = apl.tile([128, H2, 8, 8], bf16, tag='w2o8',
                                       name=pref + 'w2o')
                        _conv_pp(st, name=pref + 'w2', w=W[pref + '_w2'],
                                 bias=W[pref + '_b2'][:, 0:1], in_=w1o[:],
                                 out=w2o[:], taps=TAPS3, s=1, H=8, W=8, OH=8, OW=8)
                        out0 = apl.tile([128, BQ, 8, 8], bf16, tag='blk8', bufs=4,
                                        name=pref + 'o0')
                        out1 = apl.tile([128, BQ, 8, 8], bf16, tag='blk8', bufs=4,
                                        name=pref + 'o1')
                        if bi == 0:
                            def ex(mc, half, b0, b1, _w=W['l1b0_wd'], _in=po):
                                p0 = 64 * half
                                return [(_w[p0:p0 + 64, 0, 0, mc, :],
                                         _in[p0:p0 + 64, b0:b1], (p0, 0))]
                        else:
                            def ex(mc, half, b0, b1, _ib=W['identb'], _in=cur):
                                o = half * H2
                                return [(_ib[:], _in[mc][:, o + b0:o + b1], None)]
                        _conv_pd(st, name=pref + 'w3', w=W[pref + '_w3'],
                                 bias=W[pref + '_b3'], in_=w2o[:],
                                 outs=[out0[:], out1[:]], s=1, H=8, W=8,
                                 OH=8, OW=8, extras=ex)
                        if DBG and q == 0 and bi == 0:
                            nc.sync.dma_start(out=ddbg['dbg_w1o'][:], in_=_flat(w1o[:]))
                            nc.sync.dma_start(out=ddbg['dbg_w2o'][:], in_=_flat(w2o[:]))
                            nc.sync.dma_start(out=ddbg['dbg_o0'][:], in_=_flat(out0[:]))
                            nc.sync.dma_start(out=ddbg['dbg_o1'][:], in_=_flat(out1[:]))
                        cur = [out0[:], out1[:]]

                    # ---- layer2
                    for bi in range(4):
                        pref = f'l2b{bi}'
                        kps_in = [c.shape[0] for c in cur]
                        if bi == 0:
                            w1o = apl.tile([128, BQ, 8, 8], bf16, tag='w1o8b',
                                           name=pref + 'w1o')
                            _conv_dd(st, name=pref + 'w1', w=W[pref + '_w1'],
                                     bias=W[pref + '_b1'], kps=kps_in, ins=cur,
                                     outs=[w1o[:]], taps=TAP1, s=1,
                                     H=8, W=8, OH=8, OW=8)
                            w2o = apl.tile([128, BQ, 4, 4], bf16, tag='w2o4',
                                           name=pref + 'w2o')
                            _conv_dd(st, name=pref + 'w2', w=W[pref + '_w2'],
                                     bias=W[pref + '_b2'], kps=[128], ins=[w1o[:]],
                                     outs=[w2o[:]], taps=TAPS3, s=2,
                                     H=8, W=8, OH=4, OW=4)
                        else:
                            w1o = apl.tile([128, BQ, 4, 4], bf16, tag='w1o4',
                                           name=pref + 'w1o')
                            _conv_dd(st, name=pref + 'w1', w=W[pref + '_w1'],
                                     bias=W[pref + '_b1'], kps=kps_in, ins=cur,
                                     outs=[w1o[:]], taps=TAP1, s=1,
                                     H=4, W=4, OH=4, OW=4)
                            w2o = apl.tile([128, BQ, 4, 4], bf16, tag='w2o4',
                                           name=pref + 'w2o')
                            _conv_dd(st, name=pref + 'w2', w=W[pref + '_w2'],
                                     bias=W[pref + '_b2'], kps=[128], ins=[w1o[:]],
                                     outs=[w2o[:]], taps=TAPS3, s=1,
                                     H=4, W=4, OH=4, OW=4)
                        if bi == 3:
                            outs = [feat[i][:, q0:q0 + BQ].rearrange(
                                'p b (h w) -> p b h w', h=4) for i in range(4)]
                        else:
                            outs = [apl.tile([128, BQ, 4, 4], bf16, tag='blk4',
                                             bufs=8, name=f'{pref}o{i}')[:]
                                    for i in range(4)]
                        if bi == 0:
                            def ex(mc, b0, b1, _w=W['l2b0_wd'], _in=cur):
                                return [(_w[:_in[kc].shape[0], kc, 0, mc, :],
                                         _in[kc][:, b0:b1, 0:7:2, 0:7:2])
                                        for kc in range(len(_in))]
                        else:
                            def ex(mc, b0, b1, _ib=W['identb'], _in=cur):
                                return [(_ib[:], _in[mc][:, b0:b1])]
                        _conv_dd(st, name=pref + 'w3', w=W[pref + '_w3'],
                                 bias=W[pref + '_b3'], kps=[128], ins=[w2o[:]],
                                 outs=outs, taps=TAP1, s=1, H=4, W=4, OH=4, OW=4,
                                 extras=ex)
                        cur = outs

                # ------------ fc head + projection + loss (same pool
                # scope: PE flows from the last conv into fc while warm)
                fcp = st.fcB
                psf = [st.psA.tile([128, B], f32, tag='ps', name=f'fcps{co}')
                       for co in range(4)]
                for ci in range(4):
                    fcw = st.fcwt[ci]
                    for co in range(4):
                        for hw in range(16):
                            nc.tensor.matmul(psf[co][:], fcw[:, hw, co, :],
                                             feat[ci][:, :, hw],
                                             start=(ci == 0 and hw == 0),
                                             stop=(ci == 3 and hw == 15),
                                             skip_group_check=True)
                fsb = []
                for co in range(4):
                    ft = fcp.tile([128, B], bf16, name=f'f{co}')
                    _evac(st, psf[co][:], ft[:], W['fc_b'][:, co:co + 1], True)
                    fsb.append(ft)

                f1sb = []
                for mc in range(2):
                    ps = st.psA.tile([128, B], f32, tag='ps', name=f'f1ps{mc}')
                    for kc in range(4):
                        nc.tensor.matmul(ps[:], W['fc1_w'][:, kc, 0, mc, :],
                                         fsb[kc][:], start=(kc == 0),
                                         stop=(kc == 3))
                    f1 = fcp.tile([128, B], f32, name=f'f1_{mc}')
                    _evac(st, ps[:], f1[:], W['fc1_b'][:, mc:mc + 1], True)
                    f1sb.append(f1)

                # ---- stereographic projection + log-softmax CE (fp32)
                s_row = fcp.tile([1, B], f32, name='s_row')
                ps_loss = st.psA.tile([1, 1], f32, tag='ps', name='ps_loss')
                for bc in range(2):
                    psT = st.psA.tile([128, 257], f32, tag='ps', name=f'psT{bc}')
                    for cc in range(2):
                        nc.tensor.transpose(psT[:, cc * 128:(cc + 1) * 128],
                                            f1sb[cc][:, bc * 128:(bc + 1) * 128],
                                            W['identf'][:])
                    fT = fcp.tile([128, 256], f32, name=f'fT{bc}')
                    nc.scalar.copy(fT[:], psT[:, 0:256])
                    junk = fcp.tile([128, 257], f32, tag='junk', bufs=2,
                                    name=f'junk{bc}')
                    sq = fcp.tile([128, 1], f32, name=f'sq{bc}')
                    nc.scalar.activation(junk[:, 0:256], fT[:], AF.Square,
                                         accum_out=sq[:])
                    t1 = fcp.tile([128, 1], f32, name=f't1_{bc}')
                    nc.vector.tensor_scalar_add(t1[:], sq[:], 1.0)
                    rec = fcp.tile([128, 1], f32, name=f'rec{bc}')
                    nc.vector.reciprocal(rec[:], t1[:])
                    t2 = fcp.tile([128, 1], f32, name=f't2_{bc}')
                    nc.vector.tensor_scalar_add(t2[:], sq[:], -1.0)
                    sT = fcp.tile([128, 1], f32, name=f'sT{bc}')
                    nc.vector.tensor_mul(sT[:], t2[:], rec[:])
                    omsT = fcp.tile([128, 1], f32, name=f'omsT{bc}')
                    nc.vector.tensor_scalar(omsT[:], sT[:], -1.0, 1.0, ALU.mult,
                                            ALU.add)
                    projT = fcp.tile([128, 257], f32, tag='projT', bufs=2,
                                     name=f'projT{bc}')
                    nc.vector.tensor_scalar_mul(projT[:, 0:256], fT[:], omsT[:])
                    nc.vector.tensor_copy(projT[:, 256:257], sT[:])
                    psS = st.psA.tile([1, 128], f32, tag='ps', name=f'psS{bc}')
                    nc.tensor.transpose(psS[:], sT[:], W['identf'][:])
                    nc.scalar.copy(s_row[:, bc * 128:(bc + 1) * 128], psS[:])
                    mx = fcp.tile([128, 1], f32, name=f'mx{bc}')
                    nc.vector.tensor_reduce(mx[:], projT[:], mybir.AxisListType.X,
                                            ALU.max)
                    nmx = fcp.tile([128, 1], f32, name=f'nmx{bc}')
                    nc.vector.tensor_scalar_mul(nmx[:], mx[:], -1.0)
                    se = fcp.tile([128, 1], f32, name=f'se{bc}')
                    nc.scalar.activation(junk[:], projT[:], AF.Exp, bias=nmx[:],
                                         scale=1.0, accum_out=se[:])
                    lse = fcp.tile([128, 1], f32, name=f'lse{bc}')
                    nc.scalar.activation(lse[:], se[:], AF.Ln)
                    lse2 = fcp.tile([128, 1], f32, name=f'lse2{bc}')
                    nc.vector.tensor_add(lse2[:], lse[:], mx[:])
                    picked = fcp.tile([128, 1], f32, name=f'picked{bc}')
                    junk2 = fcp.tile([128, 257], f32, tag='junk', bufs=2,
                                     name=f'junkb{bc}')
                    nc.vector.scalar_tensor_tensor(junk2[:], projT[:], 1.0,
                                                   W['oneh'][:, bc, :], ALU.mult,
                                                   ALU.mult, accum_out=picked[:])
                    lossv = fcp.tile([128, 1], f32, name=f'lossv{bc}')
                    nc.vector.tensor_sub(lossv[:], lse2[:], picked[:])
                    nc.tensor.matmul(ps_loss[:], W['ones128'][:], lossv[:],
                                     start=(bc == 0), stop=(bc == 1))

                oms_row = fcp.tile([1, B], f32, name='oms_row')
                nc.vector.tensor_scalar(oms_row[:], s_row[:], -1.0, 1.0, ALU.mult,
                                        ALU.add)
                ps_bc = st.psA.tile([128, B], f32, tag='ps', name='ps_bc')
                nc.tensor.matmul(ps_bc[:], W['ones1x'][:], oms_row[:],
                                 start=True, stop=True)
                proj0 = fcp.tile([128, B], f32, name='proj0')
                nc.vector.tensor_mul(proj0[:], f1sb[0][:], ps_bc[:])
                proj1 = fcp.tile([128, B], f32, name='proj1')
                nc.vector.tensor_mul(proj1[:], f1sb[1][:], ps_bc[:])
                ps_out = st.psA.tile([10, B], f32, tag='ps', name='ps_out')
                nc.tensor.matmul(ps_out[:], W['fc2_w0'][:], proj0[:], start=True,
                                 stop=False)
                nc.tensor.matmul(ps_out[:], W['fc2_w1'][:], proj1[:], start=False,
                                 stop=False)
                nc.tensor.matmul(ps_out[:], W['fc2_w2'][:], s_row[:], start=False,
                                 stop=True)
                out_sb = fcp.tile([10, B], f32, name='out_sb')
                nc.scalar.copy(out_sb[:], ps_out[:])
                nc.sync.dma_start(out=d_out[:], in_=out_sb[:])
                loss_sb = fcp.tile([1, 1], f32, name='loss_sb')
                nc.scalar.copy(loss_sb[:], ps_loss[:])
                nc.sync.dma_start(out=d_loss[:], in_=loss_sb[:])

    nc.compile()
    return nc


def _get_prog():
    global _PROG
    if _PROG is None:
        _PROG = _build_program()
    return _PROG


# ---------------------------------------------------------------- entry point
def kernel(x, params, labels, _return_runner=False):
    nc = _get_prog()
    arrs = _host_arrays(params)
    per = _host_percore(x, labels)
    in_maps = [{**arrs, **per[c]} for c in range(NCORES)]

    def run():
        res = run_bass_kernel_spmd(nc, in_maps, list(range(NCORES)))
        outs = np.concatenate([res.results[c]['out_t'].T for c in range(NCORES)],
                              axis=0)
        loss = np.float32(sum(float(res.results[c]['lossp'][0, 0])
                              for c in range(NCORES)) / BTOT)
        return outs.astype(np.float32), loss

    if _return_runner:
        return run
    return run()
